# revision 1
# baseline (speedup 1.0000x reference)
"""Trainium2 Bass kernel for nn_DiscriminativeLoss.

Data-parallel over the batch axis: each of the 8 NeuronCores gets one sample
(input[b] of shape (32, 65536) plus target[b, 0] of shape (65536,)) and
computes the per-sample sufficient statistics on-chip:

  cnt0      = sum_n t0[n]
  s0[f]     = sum_n x[f,n] * t0[n]          (masked feature sums, cluster 0)
  rs[f]     = sum_n x[f,n]                  (total feature sums; s1 = rs - s0)
  m0,m1     = cluster means (safe-count divide, on-chip)
  v0        = sum_n max(||x_n - m0|| - dv, 0)^2 * t0[n]
  v1        = sum_n max(||x_n - m1|| - dv, 0)^2 * (1 - t0[n])

The host combines the 8 small per-core result vectors into the scalar loss
(the tiny all-reduce-mean step of the batch-parallel sharding).

On-chip layout (per core), n = 16384*jj + u = 512*p + q:
  X, Xsq [128, 16384] : partition (32*jj+f), free u
  T0n    [128, 512]   : partition p, free q
  T0cc   [128, 5632]  : partition (32*z+jj), free (512*g+q), chunk i = 11*z+g
Phase 1: per 512-col chunk, PE replicates t0 across the 32 f-partitions of
each quadrant (K=4 matmul from T0cc) and DVE tensor_tensor_reduce
accumulates the masked sums; row sums via ACT/DVE accumulate; squares into
Xsq split across ACT/DVE/GPSIMD. After the means are formed on-chip, phase 2
computes dist_c^2 - ||m_c||^2 directly in PSUM with two accumulating fp32r
matmuls per chunk (block -2*m_c weights over X, then block-ones over Xsq),
packing 3 chunks per PSUM bank (bases 0/32/64) so one [96,512] engine copy
evacuates 3 chunks; a SBUF->SBUF DMA re-lays the rows into the n-major
D01 [128, 1024] tile where the sqrt/hinge chain and masked reductions run.
"""

import numpy as np
from contextlib import ExitStack

BS, NF, MAXC, NLOC = 8, 32, 4, 65536
DELTA_VAR, DELTA_DIST = 0.5, 1.5
ALPHA, BETA, GAMMA = 1.0, 1.0, 1e-4

NCORES = 8
CH = 32          # 512-column chunks per core
CW = 512         # chunk width
U = NLOC // 4    # 16384 columns per quadrant
NG = 11          # chunks per z-group (CH = 3*11 - 1)

_CACHE = {}

# engine splits (tuned against trace): chunk index -> engine
RS_ENGINE = ["act"] * 26 + ["dve"] * 6
SQ_ENGINE = (["gps", "gps", "dve"] * 6 + ["gps", "act"] * 7)[:32]
EVAC_ENGINE = ["act", "dve"] * 6


def _zg(i):
    return i // NG, i % NG


def _host_constants():
    sel33 = np.zeros((128, 33), dtype=np.float32)
    for p in range(128):
        sel33[p, p % 32] = 1.0
    sel33[:, 32] = 1.0
    ones33 = np.ones((128, 33), dtype=np.float32)
    # cw1p: block-ones, col 2*jj+c (c=0,1) selects quadrant jj; cols 8..32 zero
    w1p = np.zeros((128, 32), dtype=np.float32)
    for jj in range(4):
        for c in range(2):
            w1p[32 * jj:32 * jj + 32, 2 * jj + c] = 1.0
    # replicated at partition bases 0/32/64 so lhsT base matches rhs base
    lhsT4 = np.zeros((128, 128), dtype=np.float32)
    for z in range(3):
        for jj in range(4):
            lhsT4[32 * z + jj, 32 * jj:32 * jj + 32] = 1.0
    import ml_dtypes
    cst = np.concatenate([sel33, ones33], axis=1)  # [128, 66] fp32
    cstb = np.concatenate([w1p, lhsT4], axis=1).astype(ml_dtypes.bfloat16)  # [128, 160]
    return {"cst": cst, "cstb": cstb}


def _emit(ctx, tc, x_d, t0_d, t0b_d, cst_d, cstb_d, res_d):
    import concourse.mybir as mybir

    nc = tc.nc
    f32 = mybir.dt.float32
    bf16 = mybir.dt.bfloat16
    Alu = mybir.AluOpType
    Act = mybir.ActivationFunctionType
    AxX = mybir.AxisListType.X

    persist = ctx.enter_context(tc.tile_pool(name="persist", bufs=1))
    scratch = ctx.enter_context(tc.tile_pool(name="scratch", bufs=1))
    stage_p = ctx.enter_context(tc.tile_pool(name="stage", bufs=2))
    p_t0rep = ctx.enter_context(tc.tile_pool(name="p_t0rep", bufs=2, space="PSUM"))
    p_dist = ctx.enter_context(tc.tile_pool(name="p_dist", bufs=2, space="PSUM"))
    p_fin = ctx.enter_context(tc.tile_pool(name="p_fin", bufs=2, space="PSUM"))

    def ptile(shape, tag, dtype=f32):
        return persist.tile(shape, dtype, tag=tag, name=tag)

    # ---- persistent tiles ----
    X = ptile([128, U], "X")
    Xb = ptile([128, U], "Xb", dtype=bf16)
    Xsqb = ptile([128, U], "Xsqb", dtype=bf16)
    T0ccb = ptile([128, NG * CW], "T0ccb", dtype=bf16)
    # masks in the D01 partition layout: P = 4*i + jj, value t0[16384*jj+512*i+q]
    TT01 = ptile([128, 2 * CW], "TT01")    # [T0n | T1n]
    T0n = TT01[:, 0:CW]
    T1n = TT01[:, CW:2 * CW]
    D01 = ptile([128, 2 * CW], "D01")
    CST = ptile([128, 66], "CST")          # [csel | cone] fp32
    csel = CST[:, 0:33]
    cone = CST[:, 33:66]
    CSTB = ptile([128, 160], "CSTB", dtype=bf16)   # [cw1p | clhst4] bf16
    cw1p = CSTB[:, 0:32]
    clhst4 = CSTB[:, 32:160]
    W2B = ptile([128, 32], "W2B", dtype=bf16)
    mnegb = ptile([32, 2], "mnegb", dtype=bf16)
    s0pc = ptile([128, CH], "s0pc")
    rspc = ptile([128, CH], "rspc")
    MISC = ptile([128, 64], "MISC")
    stats3 = MISC[:, 32:35]
    vstats = MISC[:, 35:37]
    stat_f = MISC[0:33, 37:40]
    stat_t = MISC[0:33, 40:43]
    cnts = MISC[0:32, 43:47]               # cols: cnt0s, cnt1s, rc0, rc1
    mraw = MISC[0:32, 47:49]
    mneg = MISC[0:32, 49:51]
    csb = MISC[0:1, 51:53]
    vout = MISC[0:1, 53:55]
    bias01 = MISC[:, 55:57]
    biasdv = MISC[:, 57:58]
    s1col = MISC[0:32, 58:59]

    # ---- loads ----
    x_ap = x_d.ap()
    t0_ap = t0_d.ap()
    nc.gpsimd.memset(biasdv, -DELTA_VAR)
    nc.sync.dma_start(T0n, t0_ap.rearrange("(jj i q) -> i jj q", jj=4, i=32))
    # T0ccb[32*z+jj, 512*g+q] = t0[16384*jj + 512*(11*z+g) + q]  (bf16, exact)
    t0b_ap = t0b_d.ap()
    for z in range(3):
        span = NG * CW if z < 2 else (CH - 2 * NG) * CW
        for jj in range(4):
            nc.scalar.dma_start(
                T0ccb[32 * z + jj:32 * z + jj + 1, 0:span],
                t0b_ap[U * jj + NG * CW * z: U * jj + NG * CW * z + span],
            )
    nc.sync.dma_start(CST[:], cst_d.ap())
    nc.sync.dma_start(CSTB[:], cstb_d.ap())
    # column-major windows so phase-1 chunks can start as soon as their
    # window (all 4 quadrants) has landed; spread across the 3 DGE queues
    dma_engines = [nc.sync, nc.scalar]
    WW = 2048
    for w in range(U // WW):
        for jj in range(4):
            dma_engines[jj % 2].dma_start(
                X[32 * jj:32 * jj + 32, w * WW:(w + 1) * WW],
                x_ap[:, jj * U + w * WW: jj * U + (w + 1) * WW],
            )

    # ---- phase 1 ----
    for i in range(CH):
        z, g = _zg(i)
        xs = X[:, i * CW:(i + 1) * CW]
        t0rep = p_t0rep.tile([128, CW], f32, tag="t0rep")
        nc.tensor.matmul(
            t0rep[:], clhst4[32 * z:32 * z + 4, :],
            T0ccb[32 * z:32 * z + 4, g * CW:(g + 1) * CW],
            start=True, stop=True,
        )
        # custom DVE ops (ttr) cannot read PSUM on HW: plain TT then reduce
        scr = scratch.tile([128, CW], f32, tag="scr_ttr", name="scr")
        nc.vector.tensor_tensor(out=scr[:], in0=xs, in1=t0rep[:], op=Alu.mult)
        nc.vector.reduce_sum(s0pc[:, i:i + 1], scr[:], axis=AxX)
    # ---- rs cast + squares in 2048-wide chunks (4x fewer ops) ----
    BW = 4 * CW
    for w in range(U // BW):
        xs = X[:, w * BW:(w + 1) * BW]
        # rs + bf16 cast fused: ACT copy X -> Xb with accumulate
        nc.scalar.activation(
            out=Xb[:, w * BW:(w + 1) * BW], in_=xs, func=Act.Copy,
            accum_out=rspc[:, w:w + 1])
        xq = Xsqb[:, w * BW:(w + 1) * BW]
        eng = ["gps", "dve", "gps", "act", "gps", "dve", "gps", "act"][w]
        if eng == "act":
            nc.scalar.activation(out=xq, in_=xs, func=Act.Square)
        elif eng == "dve":
            nc.vector.tensor_tensor(out=xq, in0=xs, in1=xs, op=Alu.mult)
        else:
            nc.gpsimd.tensor_tensor(out=xq, in0=xs, in1=xs, op=Alu.mult)

    # ---- cross-partition finish #1 (per-f sums + totals) ----
    nc.vector.reduce_sum(stats3[:, 0:1], s0pc[:], axis=AxX)
    nc.vector.reduce_sum(stats3[:, 1:2], rspc[:, 0:8], axis=AxX)
    nc.vector.reduce_sum(stats3[:, 2:3], T0n, axis=AxX)
    F1f = p_fin.tile([33, 3], f32, tag="fin")
    nc.tensor.matmul(F1f[:], csel, stats3, start=True, stop=True)
    F1t = p_fin.tile([33, 3], f32, tag="fin")
    nc.tensor.matmul(F1t[:], cone, stats3, start=True, stop=True)
    nc.scalar.copy(stat_f[:], F1f[:])
    nc.scalar.copy(stat_t[:], F1t[:])

    # ---- means (f on partitions 0..31; totals available on every row) ----
    cnt0col = stat_t[0:32, 2:3]
    nc.vector.tensor_scalar(
        out=cnts[:, 0:1], in0=cnt0col, scalar1=1.0, scalar2=None, op0=Alu.max)
    nc.vector.tensor_scalar(
        out=cnts[:, 1:2], in0=cnt0col, scalar1=-1.0, scalar2=float(NLOC),
        op0=Alu.mult, op1=Alu.add)
    nc.vector.tensor_scalar(
        out=cnts[:, 1:2], in0=cnts[:, 1:2], scalar1=1.0, scalar2=None, op0=Alu.max)
    nc.vector.reciprocal(cnts[:, 2:3], cnts[:, 0:1])
    nc.vector.reciprocal(cnts[:, 3:4], cnts[:, 1:2])
    nc.vector.tensor_tensor(
        out=mraw[:, 0:1], in0=stat_f[0:32, 0:1], in1=cnts[:, 2:3], op=Alu.mult)
    nc.vector.tensor_tensor(
        out=s1col, in0=stat_f[0:32, 1:2], in1=stat_f[0:32, 0:1], op=Alu.subtract)
    nc.vector.tensor_tensor(
        out=mraw[:, 1:2], in0=s1col, in1=cnts[:, 3:4], op=Alu.mult)
    nc.vector.tensor_scalar(
        out=mneg, in0=mraw, scalar1=-2.0, scalar2=None, op0=Alu.mult)

    # W2B: block-diagonal -2*m_c weights (bf16) at cols 2*jj+c (others zero)
    nc.vector.tensor_copy(mnegb, mneg)
    nc.gpsimd.memset(W2B[:], 0.0)
    for jj in range(4):
        nc.sync.dma_start(W2B[32 * jj:32 * jj + 32, 2 * jj:2 * jj + 2], mnegb)

    # ||m_c||^2 -> bias01 on all partitions
    mm0 = p_fin.tile([1, 2], f32, tag="fin")
    nc.tensor.matmul(mm0[:], mraw[:, 0:1], mraw, start=True, stop=True)
    mm1 = p_fin.tile([1, 2], f32, tag="fin")
    nc.tensor.matmul(mm1[:], mraw[:, 1:2], mraw, start=True, stop=True)
    nc.scalar.copy(csb[0:1, 0:1], mm0[0:1, 0:1])
    nc.scalar.copy(csb[0:1, 1:2], mm1[0:1, 1:2])
    nc.gpsimd.partition_broadcast(bias01[:], csb, channels=128)

    res_ap = res_d.ap()
    nc.sync.dma_start(res_ap[0:64].rearrange("(c f) -> f c", c=2), mraw)

    # ---- phase 2: dist_c^2 - ||m_c||^2 into PSUM, 3 chunks per bank ----
    for g in range(NG):
        nz = 3 if g < CH - 2 * NG else 2
        pd = p_dist.tile([128, CW], f32, tag="dist")
        for z in range(nz):
            i = NG * z + g
            nc.tensor.matmul(
                pd[32 * z:32 * z + 32, :], W2B[:],
                Xb[:, i * CW:(i + 1) * CW], start=True, stop=False)
            nc.tensor.matmul(
                pd[32 * z:32 * z + 32, :], cw1p,
                Xsqb[:, i * CW:(i + 1) * CW], start=False, stop=True)
        stg = stage_p.tile([128, CW], f32, tag="stg")
        if EVAC_ENGINE[g % len(EVAC_ENGINE)] == "act":
            nc.scalar.copy(stg[0:32 * nz, :], pd[0:32 * nz, :])
        else:
            nc.vector.tensor_copy(stg[0:32 * nz, :], pd[0:32 * nz, :])
        # widen into D01 with partition P = 4*i + jj (i = 11*z + g):
        # D01[4*i + jj, 512*c + q] = stg[32*z + 2*jj + c, q]; contiguous dst
        for z in range(nz):
            i = NG * z + g
            dst = D01[4 * i:4 * i + 4, :].rearrange("p (c q) -> p c q", c=2)
            (nc.sync if (g + z) % 2 == 0 else nc.scalar).dma_start(
                dst, stg[32 * z:32 * z + 8, :])

    # ---- hinge chain on D01 (in place) ----
    nc.vector.tensor_scalar(
        out=T1n, in0=T0n, scalar1=-1.0, scalar2=1.0,
        op0=Alu.mult, op1=Alu.add)
    for c in range(2):
        half = D01[:, c * CW:(c + 1) * CW]
        nc.vector.tensor_scalar(
            out=half, in0=half, scalar1=bias01[:, c:c + 1], scalar2=0.0,
            op0=Alu.add, op1=Alu.max)
    nc.scalar.activation(out=D01, in_=D01, func=Act.Sqrt)
    nc.scalar.activation(out=D01, in_=D01, func=Act.Relu, bias=biasdv[:, 0:1])
    nc.scalar.activation(out=D01, in_=D01, func=Act.Square)
    for c in range(2):
        scr = scratch.tile([128, CW], f32, tag="scr_ttr", name="scr")
        nc.vector.tensor_tensor(
            out=scr[:], in0=D01[:, c * CW:(c + 1) * CW],
            in1=(T0n if c == 0 else T1n), op=Alu.mult)
        nc.vector.reduce_sum(vstats[:, c:c + 1], scr[:], axis=AxX)

    # ---- final sums + outputs ----
    F2 = p_fin.tile([1, 2], f32, tag="fin")
    nc.tensor.matmul(F2[:], cone[:, 0:1], vstats, start=True, stop=True)
    nc.scalar.copy(vout, F2[0:1, 0:2])
    nc.sync.dma_start(res_ap[64:67], stat_t[0:1, 0:3])
    nc.sync.dma_start(res_ap[67:69], vout[0:1, 0:2])
    nc.sync.dma_start(res_ap[69:71], csb[0:1, 0:2])


def _build():
    import concourse.bacc as bacc
    import concourse.tile as tile
    import concourse.mybir as mybir

    f32 = mybir.dt.float32
    nc = bacc.Bacc("TRN2", target_bir_lowering=False, debug=False)
    x_d = nc.dram_tensor("x", [NF, NLOC], f32, kind="ExternalInput")
    t0_d = nc.dram_tensor("t0", [NLOC], f32, kind="ExternalInput")
    t0b_d = nc.dram_tensor("t0b", [NLOC], mybir.dt.bfloat16, kind="ExternalInput")
    cst_d = nc.dram_tensor("cst", [128, 66], f32, kind="ExternalInput")
    cstb_d = nc.dram_tensor("cstb", [128, 160], mybir.dt.bfloat16,
                            kind="ExternalInput")
    res_d = nc.dram_tensor("res", [128], f32, kind="ExternalOutput")
    with tile.TileContext(nc) as tc:
        with ExitStack() as ctx:
            _emit(ctx, tc, x_d, t0_d, t0b_d, cst_d, cstb_d, res_d)
    nc.compile()
    return nc


def get_nc():
    if "nc" not in _CACHE:
        _CACHE["nc"] = _build()
    return _CACHE["nc"]


def make_in_maps(input, target):
    consts = _host_constants()
    in_maps = []
    for b in range(input.shape[0]):
        import ml_dtypes
        t0 = np.ascontiguousarray(target[b, 0], dtype=np.float32)
        m = {
            "x": np.ascontiguousarray(input[b], dtype=np.float32),
            "t0": t0,
            "t0b": t0.astype(ml_dtypes.bfloat16),
        }
        m.update(consts)
        in_maps.append(m)
    return in_maps


def combine_host(results, n_clusters):
    """results: list of 8 dicts with 'res' vectors. Returns scalar loss."""
    total = 0.0
    for b in range(BS):
        r = np.asarray(results[b]["res"], dtype=np.float64)
        m0, m1 = r[0:32], r[32:64]
        cnt0 = r[66]
        v0, v1 = r[67], r[68]
        ncb = float(n_clusters[b])
        counts = np.array([cnt0, NLOC - cnt0])
        active = counts > 0
        safe = np.where(active, counts, 1.0)
        c_var = float(np.where(active, np.array([v0, v1]) / safe, 0.0).sum())
        l_var = c_var / ncb
        dn = float(np.sqrt(((m0 - m1) ** 2).sum()))
        c_dist = 2.0 * max(2.0 * DELTA_DIST - dn, 0.0) ** 2
        l_dist = c_dist / (2.0 * ncb * (ncb - 1.0))
        l_reg = 0.5 * (np.sqrt((m0 ** 2).sum()) + np.sqrt((m1 ** 2).sum()))
        total += ALPHA * l_var + BETA * l_dist + GAMMA * l_reg
    return np.float32(total / BS)


def kernel(input, target, n_clusters):
    from concourse import bass_utils

    nc = get_nc()
    in_maps = make_in_maps(np.asarray(input), np.asarray(target))
    br = bass_utils.run_bass_kernel_spmd(nc, in_maps, core_ids=list(range(NCORES)))
    loss = combine_host(br.results, np.asarray(n_clusters))
    return np.array(loss, dtype=np.float32)



# revision 11
# speedup vs baseline: 1.0873x; 1.0873x over previous
"""Trainium2 Bass kernel for nn_DiscriminativeLoss.

Data-parallel over the batch axis: each of the 8 NeuronCores gets one sample
(input[b] of shape (32, 65536) plus target[b, 0] of shape (65536,)) and
computes the per-sample sufficient statistics on-chip:

  cnt0      = sum_n t0[n]
  s0[f]     = sum_n x[f,n] * t0[n]          (masked feature sums, cluster 0)
  rs[f]     = sum_n x[f,n]                  (total feature sums; s1 = rs - s0)
  m0,m1     = cluster means (safe-count divide, on-chip)
  v0        = sum_n max(||x_n - m0|| - dv, 0)^2 * t0[n]
  v1        = sum_n max(||x_n - m1|| - dv, 0)^2 * (1 - t0[n])

The host combines the 8 small per-core result vectors into the scalar loss
(the tiny all-reduce-mean step of the batch-parallel sharding).

On-chip layout (per core), n = 16384*jj + u = 512*p + q:
  X, Xsq [128, 16384] : partition (32*jj+f), free u
  T0n    [128, 512]   : partition p, free q
  T0cc   [128, 5632]  : partition (32*z+jj), free (512*g+q), chunk i = 11*z+g
Phase 1: per 512-col chunk, PE replicates t0 across the 32 f-partitions of
each quadrant (K=4 matmul from T0cc) and DVE tensor_tensor_reduce
accumulates the masked sums; row sums via ACT/DVE accumulate; squares into
Xsq split across ACT/DVE/GPSIMD. After the means are formed on-chip, phase 2
computes dist_c^2 - ||m_c||^2 directly in PSUM with two accumulating fp32r
matmuls per chunk (block -2*m_c weights over X, then block-ones over Xsq),
packing 3 chunks per PSUM bank (bases 0/32/64) so one [96,512] engine copy
evacuates 3 chunks; a SBUF->SBUF DMA re-lays the rows into the n-major
D01 [128, 1024] tile where the sqrt/hinge chain and masked reductions run.
"""

import numpy as np
from contextlib import ExitStack

BS, NF, MAXC, NLOC = 8, 32, 4, 65536
DELTA_VAR, DELTA_DIST = 0.5, 1.5
ALPHA, BETA, GAMMA = 1.0, 1.0, 1e-4

NCORES = 8
CH = 32          # 512-column chunks per core
CW = 512         # chunk width
U = NLOC // 4    # 16384 columns per quadrant
NG = 11          # chunks per z-group (CH = 3*11 - 1)

_CACHE = {}

# engine splits (tuned against trace): chunk index -> engine
RS_ENGINE = ["act"] * 26 + ["dve"] * 6
SQ_ENGINE = (["gps", "gps", "dve"] * 6 + ["gps", "act"] * 7)[:32]
EVAC_ENGINE = ["act", "dve"] * 6


def _zg(i):
    return i // NG, i % NG


def _host_constants():
    sel33 = np.zeros((128, 33), dtype=np.float32)
    for p in range(128):
        sel33[p, p % 32] = 1.0
    sel33[:, 32] = 1.0
    ones33 = np.ones((128, 33), dtype=np.float32)
    # cw1p: block-ones, col 2*jj+c (c=0,1) selects quadrant jj; cols 8..32 zero
    w1p = np.zeros((128, 32), dtype=np.float32)
    for jj in range(4):
        for c in range(2):
            w1p[32 * jj:32 * jj + 32, 2 * jj + c] = 1.0
    # replicated at partition bases 0/32/64 so lhsT base matches rhs base
    lhsT4 = np.zeros((128, 128), dtype=np.float32)
    for z in range(3):
        for jj in range(4):
            lhsT4[32 * z + jj, 32 * jj:32 * jj + 32] = 1.0
    import ml_dtypes
    cst = np.concatenate([sel33, ones33], axis=1)  # [128, 66] fp32
    cstb = np.concatenate([w1p, lhsT4], axis=1).astype(ml_dtypes.bfloat16)  # [128, 160]
    return {"cst": cst, "cstb": cstb}


def _emit(ctx, tc, x_d, t0_d, t0b_d, cst_d, cstb_d, res_d):
    import concourse.mybir as mybir

    nc = tc.nc
    f32 = mybir.dt.float32
    bf16 = mybir.dt.bfloat16
    Alu = mybir.AluOpType
    Act = mybir.ActivationFunctionType
    AxX = mybir.AxisListType.X

    persist = ctx.enter_context(tc.tile_pool(name="persist", bufs=1))
    scratch = ctx.enter_context(tc.tile_pool(name="scratch", bufs=1))
    stage_p = ctx.enter_context(tc.tile_pool(name="stage", bufs=2))
    p_t0rep = ctx.enter_context(tc.tile_pool(name="p_t0rep", bufs=2, space="PSUM"))
    p_dist = ctx.enter_context(tc.tile_pool(name="p_dist", bufs=2, space="PSUM"))
    p_fin = ctx.enter_context(tc.tile_pool(name="p_fin", bufs=2, space="PSUM"))

    def ptile(shape, tag, dtype=f32):
        return persist.tile(shape, dtype, tag=tag, name=tag)

    # ---- persistent tiles ----
    X = ptile([128, U], "X")
    Xb = ptile([128, U], "Xb", dtype=bf16)
    Xsqb = ptile([128, U], "Xsqb", dtype=bf16)
    T0ccb = ptile([128, NG * CW], "T0ccb", dtype=bf16)
    # masks in the D01 partition layout: P = 4*i + jj, value t0[16384*jj+512*i+q]
    TT01 = ptile([128, 2 * CW], "TT01")    # [T0n | T1n]
    T0n = TT01[:, 0:CW]
    T1n = TT01[:, CW:2 * CW]
    D01 = ptile([128, 2 * CW], "D01")
    CST = ptile([128, 66], "CST")          # [csel | cone] fp32
    csel = CST[:, 0:33]
    cone = CST[:, 33:66]
    CSTB = ptile([128, 160], "CSTB", dtype=bf16)   # [cw1p | clhst4] bf16
    cw1p = CSTB[:, 0:32]
    clhst4 = CSTB[:, 32:160]
    W2B = ptile([128, 32], "W2B", dtype=bf16)
    mnegb = ptile([32, 2], "mnegb", dtype=bf16)
    s0pc = ptile([128, CH], "s0pc")
    rspc = ptile([128, CH], "rspc")
    MISC = ptile([128, 64], "MISC")
    stats3 = MISC[:, 32:35]
    vstats = MISC[:, 35:37]
    stat_f = MISC[0:33, 37:40]
    stat_t = MISC[0:33, 40:43]
    cnts = MISC[0:32, 43:47]               # cols: cnt0s, cnt1s, rc0, rc1
    mraw = MISC[0:32, 47:49]
    mneg = MISC[0:32, 49:51]
    csb = MISC[0:1, 51:53]
    vout = MISC[0:1, 53:55]
    bias01 = MISC[:, 55:57]
    biasdv = MISC[:, 57:58]
    s1col = MISC[0:32, 58:59]

    # ---- loads ----
    x_ap = x_d.ap()
    t0_ap = t0_d.ap()
    nc.gpsimd.memset(biasdv, -DELTA_VAR)
    nc.sync.dma_start(T0n, t0_ap.rearrange("(jj i q) -> i jj q", jj=4, i=32))
    # T0ccb[32*z+jj, 512*g+q] = t0[16384*jj + 512*(11*z+g) + q]  (bf16, exact)
    t0b_ap = t0b_d.ap()
    t0r = t0b_ap.rearrange("(jj u) -> jj u", jj=4)
    for z in range(3):
        span = NG * CW if z < 2 else (CH - 2 * NG) * CW
        nc.scalar.dma_start(
            T0ccb[32 * z:32 * z + 4, 0:span],
            t0r[:, NG * CW * z: NG * CW * z + span],
        )
    nc.sync.dma_start(CST[:], cst_d.ap())
    nc.sync.dma_start(CSTB[:], cstb_d.ap())
    # X in 8 window DMAs [128, 2048] (partition (jj f), cols u-window):
    # one HWDGE hold each instead of 4, alternating the two HWDGE queues
    xr = x_ap.rearrange("f (jj u) -> jj f u", jj=4)
    BW = 4 * CW
    for w in range(U // BW):
        (nc.sync if w % 2 == 0 else nc.scalar).dma_start(
            X[:, w * BW:(w + 1) * BW], xr[:, :, w * BW:(w + 1) * BW])

    # ---- phase 1: per 2048 window ----
    # t0rep replicated across a 4-bank PSUM tile by 4 PE matmuls, then ONE
    # fused multiply+accumulate (scalar_tensor_tensor) gives the masked sum
    # s0 partial; ACT does rs+bf16 cast fused; squares from Xb (bf16 2x).
    for i in range(CH):
        z, g = _zg(i)
        xs = X[:, i * CW:(i + 1) * CW]
        t0rep = p_t0rep.tile([128, CW], f32, tag="t0rep")
        nc.tensor.matmul(
            t0rep[:], clhst4[32 * z:32 * z + 4, :],
            T0ccb[32 * z:32 * z + 4, g * CW:(g + 1) * CW],
            start=True, stop=True,
        )
        scr = scratch.tile([128, CW], f32, tag="scr_ttr", name="scr")
        nc.vector.tensor_tensor(out=scr[:], in0=xs, in1=t0rep[:], op=Alu.mult)
        nc.vector.reduce_sum(s0pc[:, i:i + 1], scr[:], axis=AxX)
    for w in range(U // BW):
        xs = X[:, w * BW:(w + 1) * BW]
        # rs + bf16 cast fused: ACT copy X -> Xb with accumulate
        nc.scalar.activation(
            out=Xb[:, w * BW:(w + 1) * BW], in_=xs, func=Act.Copy,
            accum_out=rspc[:, w:w + 1])
        xq = Xsqb[:, w * BW:(w + 1) * BW]
        eng = ["gps", "dve", "gps", "act", "gps", "dve", "gps", "act"][w]
        if eng == "act":
            nc.scalar.activation(out=xq, in_=xs, func=Act.Square)
        elif eng == "dve":
            nc.vector.tensor_tensor(out=xq, in0=xs, in1=xs, op=Alu.mult)
        else:
            nc.gpsimd.tensor_tensor(out=xq, in0=xs, in1=xs, op=Alu.mult)

    # ---- cross-partition finish #1 (per-f sums + totals) ----
    nc.vector.reduce_sum(stats3[:, 0:1], s0pc[:], axis=AxX)
    nc.vector.reduce_sum(stats3[:, 1:2], rspc[:, 0:8], axis=AxX)
    nc.vector.reduce_sum(stats3[:, 2:3], T0n, axis=AxX)
    F1f = p_fin.tile([33, 3], f32, tag="fin")
    nc.tensor.matmul(F1f[:], csel, stats3, start=True, stop=True)
    F1t = p_fin.tile([33, 3], f32, tag="fin")
    nc.tensor.matmul(F1t[:], cone, stats3, start=True, stop=True)
    nc.scalar.copy(stat_f[:], F1f[:])
    nc.scalar.copy(stat_t[:], F1t[:])

    # ---- means (f on partitions 0..31; totals available on every row) ----
    cnt0col = stat_t[0:32, 2:3]
    nc.vector.tensor_scalar(
        out=cnts[:, 0:1], in0=cnt0col, scalar1=1.0, scalar2=None, op0=Alu.max)
    nc.vector.tensor_scalar(
        out=cnts[:, 1:2], in0=cnt0col, scalar1=-1.0, scalar2=float(NLOC),
        op0=Alu.mult, op1=Alu.add)
    nc.vector.tensor_scalar(
        out=cnts[:, 1:2], in0=cnts[:, 1:2], scalar1=1.0, scalar2=None, op0=Alu.max)
    nc.vector.reciprocal(cnts[:, 2:3], cnts[:, 0:1])
    nc.vector.reciprocal(cnts[:, 3:4], cnts[:, 1:2])
    nc.vector.tensor_tensor(
        out=mraw[:, 0:1], in0=stat_f[0:32, 0:1], in1=cnts[:, 2:3], op=Alu.mult)
    nc.vector.tensor_tensor(
        out=s1col, in0=stat_f[0:32, 1:2], in1=stat_f[0:32, 0:1], op=Alu.subtract)
    nc.vector.tensor_tensor(
        out=mraw[:, 1:2], in0=s1col, in1=cnts[:, 3:4], op=Alu.mult)
    nc.vector.tensor_scalar(
        out=mneg, in0=mraw, scalar1=-2.0, scalar2=None, op0=Alu.mult)

    # W2B: block-diagonal -2*m_c weights (bf16) at cols 2*jj+c (others zero)
    nc.vector.tensor_copy(mnegb, mneg)
    nc.gpsimd.memset(W2B[:], 0.0)
    for jj in range(4):
        nc.sync.dma_start(W2B[32 * jj:32 * jj + 32, 2 * jj:2 * jj + 2], mnegb)

    # ||m_c||^2 -> bias01 on all partitions
    mm0 = p_fin.tile([1, 2], f32, tag="fin")
    nc.tensor.matmul(mm0[:], mraw[:, 0:1], mraw, start=True, stop=True)
    mm1 = p_fin.tile([1, 2], f32, tag="fin")
    nc.tensor.matmul(mm1[:], mraw[:, 1:2], mraw, start=True, stop=True)
    nc.scalar.copy(csb[0:1, 0:1], mm0[0:1, 0:1])
    nc.scalar.copy(csb[0:1, 1:2], mm1[0:1, 1:2])
    nc.gpsimd.partition_broadcast(bias01[:], csb, channels=128)

    res_ap = res_d.ap()
    nc.sync.dma_start(res_ap[0:64].rearrange("(c f) -> f c", c=2), mraw)

    # ---- phase 2: dist_c^2 - ||m_c||^2 into PSUM, 3 chunks per bank ----
    d01v = D01.rearrange("(i p) (c q) -> i p c q", i=32, c=2)
    for g in range(NG):
        nz = 3 if g < CH - 2 * NG else 2
        pd = p_dist.tile([128, CW], f32, tag="dist")
        for z in range(nz):
            i = NG * z + g
            nc.tensor.matmul(
                pd[32 * z:32 * z + 32, :], W2B[:],
                Xb[:, i * CW:(i + 1) * CW], start=True, stop=False)
            nc.tensor.matmul(
                pd[32 * z:32 * z + 32, :], cw1p,
                Xsqb[:, i * CW:(i + 1) * CW], start=False, stop=True)
        stg = stage_p.tile([128, CW], f32, tag="stg")
        if EVAC_ENGINE[g % len(EVAC_ENGINE)] == "act":
            nc.scalar.copy(stg[0:32 * nz, :], pd[0:32 * nz, :])
        else:
            nc.vector.tensor_copy(stg[0:32 * nz, :], pd[0:32 * nz, :])
        # widen into D01 with partition P = 4*i + jj (i = 11*z + g):
        # D01[4*i + jj, 512*c + q] = stg[32*z + 2*jj + c, q]; contiguous dst
        for z in range(nz):
            i = NG * z + g
            dst = D01[4 * i:4 * i + 4, :].rearrange("p (c q) -> p c q", c=2)
            (nc.sync if (g + z) % 2 == 0 else nc.scalar).dma_start(
                dst, stg[32 * z:32 * z + 8, :])

    # ---- hinge chain on D01 (in place) ----
    nc.vector.tensor_scalar(
        out=T1n, in0=T0n, scalar1=-1.0, scalar2=1.0,
        op0=Alu.mult, op1=Alu.add)
    for c in range(2):
        half = D01[:, c * CW:(c + 1) * CW]
        nc.vector.tensor_scalar(
            out=half, in0=half, scalar1=bias01[:, c:c + 1], scalar2=0.0,
            op0=Alu.add, op1=Alu.max)
    nc.scalar.activation(out=D01, in_=D01, func=Act.Sqrt)
    nc.scalar.activation(out=D01, in_=D01, func=Act.Relu, bias=biasdv[:, 0:1])
    nc.scalar.activation(out=D01, in_=D01, func=Act.Square)
    for c in range(2):
        scr2 = scratch.tile([128, CW], f32, tag="scr_tail", name="scr2")
        nc.vector.tensor_tensor(
            out=scr2[:], in0=D01[:, c * CW:(c + 1) * CW],
            in1=(T0n if c == 0 else T1n), op=Alu.mult)
        nc.vector.reduce_sum(vstats[:, c:c + 1], scr2[:], axis=AxX)

    # ---- final sums + outputs ----
    F2 = p_fin.tile([1, 2], f32, tag="fin")
    nc.tensor.matmul(F2[:], cone[:, 0:1], vstats, start=True, stop=True)
    nc.scalar.copy(vout, F2[0:1, 0:2])
    nc.sync.dma_start(res_ap[64:67], stat_t[0:1, 0:3])
    nc.sync.dma_start(res_ap[67:69], vout[0:1, 0:2])
    nc.sync.dma_start(res_ap[69:71], csb[0:1, 0:2])


def _build():
    import concourse.bacc as bacc
    import concourse.tile as tile
    import concourse.mybir as mybir

    f32 = mybir.dt.float32
    nc = bacc.Bacc("TRN2", target_bir_lowering=False, debug=False)
    x_d = nc.dram_tensor("x", [NF, NLOC], f32, kind="ExternalInput")
    t0_d = nc.dram_tensor("t0", [NLOC], f32, kind="ExternalInput")
    t0b_d = nc.dram_tensor("t0b", [NLOC], mybir.dt.bfloat16, kind="ExternalInput")
    cst_d = nc.dram_tensor("cst", [128, 66], f32, kind="ExternalInput")
    cstb_d = nc.dram_tensor("cstb", [128, 160], mybir.dt.bfloat16,
                            kind="ExternalInput")
    res_d = nc.dram_tensor("res", [128], f32, kind="ExternalOutput")
    with tile.TileContext(nc) as tc:
        with ExitStack() as ctx:
            _emit(ctx, tc, x_d, t0_d, t0b_d, cst_d, cstb_d, res_d)
    nc.compile()
    return nc


def get_nc():
    if "nc" not in _CACHE:
        _CACHE["nc"] = _build()
    return _CACHE["nc"]


def make_in_maps(input, target):
    consts = _host_constants()
    in_maps = []
    for b in range(input.shape[0]):
        import ml_dtypes
        t0 = np.ascontiguousarray(target[b, 0], dtype=np.float32)
        m = {
            "x": np.ascontiguousarray(input[b], dtype=np.float32),
            "t0": t0,
            "t0b": t0.astype(ml_dtypes.bfloat16),
        }
        m.update(consts)
        in_maps.append(m)
    return in_maps


def combine_host(results, n_clusters):
    """results: list of 8 dicts with 'res' vectors. Returns scalar loss."""
    total = 0.0
    for b in range(BS):
        r = np.asarray(results[b]["res"], dtype=np.float64)
        m0, m1 = r[0:32], r[32:64]
        cnt0 = r[66]
        v0, v1 = r[67], r[68]
        ncb = float(n_clusters[b])
        counts = np.array([cnt0, NLOC - cnt0])
        active = counts > 0
        safe = np.where(active, counts, 1.0)
        c_var = float(np.where(active, np.array([v0, v1]) / safe, 0.0).sum())
        l_var = c_var / ncb
        dn = float(np.sqrt(((m0 - m1) ** 2).sum()))
        c_dist = 2.0 * max(2.0 * DELTA_DIST - dn, 0.0) ** 2
        l_dist = c_dist / (2.0 * ncb * (ncb - 1.0))
        l_reg = 0.5 * (np.sqrt((m0 ** 2).sum()) + np.sqrt((m1 ** 2).sum()))
        total += ALPHA * l_var + BETA * l_dist + GAMMA * l_reg
    return np.float32(total / BS)


def kernel(input, target, n_clusters):
    from concourse import bass_utils

    nc = get_nc()
    in_maps = make_in_maps(np.asarray(input), np.asarray(target))
    br = bass_utils.run_bass_kernel_spmd(nc, in_maps, core_ids=list(range(NCORES)))
    loss = combine_host(br.results, np.asarray(n_clusters))
    return np.array(loss, dtype=np.float32)



# revision 18
# speedup vs baseline: 1.1498x; 1.0575x over previous
"""Trainium2 Bass kernel for nn_DiscriminativeLoss.

Data-parallel over the batch axis: each of the 8 NeuronCores gets one sample
(input[b] of shape (32, 65536) plus target[b, 0] of shape (65536,)) and
computes the per-sample sufficient statistics on-chip:

  cnt0      = sum_n t0[n]
  s0[f]     = sum_n x[f,n] * t0[n]          (masked feature sums, cluster 0)
  rs[f]     = sum_n x[f,n]                  (total feature sums; s1 = rs - s0)
  m0,m1     = cluster means (safe-count divide, on-chip)
  v0        = sum_n max(||x_n - m0|| - dv, 0)^2 * t0[n]
  v1        = sum_n max(||x_n - m1|| - dv, 0)^2 * (1 - t0[n])

The host combines the 8 small per-core result vectors into the scalar loss
(the tiny all-reduce-mean step of the batch-parallel sharding).

On-chip layout (per core), n = 16384*jj + u = 512*p + q:
  X, Xsq [128, 16384] : partition (32*jj+f), free u
  T0n    [128, 512]   : partition p, free q
  T0cc   [128, 5632]  : partition (32*z+jj), free (512*g+q), chunk i = 11*z+g
Phase 1: per 512-col chunk, PE replicates t0 across the 32 f-partitions of
each quadrant (K=4 matmul from T0cc) and DVE tensor_tensor_reduce
accumulates the masked sums; row sums via ACT/DVE accumulate; squares into
Xsq split across ACT/DVE/GPSIMD. After the means are formed on-chip, phase 2
computes dist_c^2 - ||m_c||^2 directly in PSUM with two accumulating fp32r
matmuls per chunk (block -2*m_c weights over X, then block-ones over Xsq),
packing 3 chunks per PSUM bank (bases 0/32/64) so one [96,512] engine copy
evacuates 3 chunks; a SBUF->SBUF DMA re-lays the rows into the n-major
D01 [128, 1024] tile where the sqrt/hinge chain and masked reductions run.
"""

import numpy as np
from contextlib import ExitStack

BS, NF, MAXC, NLOC = 8, 32, 4, 65536
DELTA_VAR, DELTA_DIST = 0.5, 1.5
ALPHA, BETA, GAMMA = 1.0, 1.0, 1e-4

NCORES = 8
CH = 32          # 512-column chunks per core
CW = 512         # chunk width
U = NLOC // 4    # 16384 columns per quadrant
NG = 11          # chunks per z-group (CH = 3*11 - 1)

_CACHE = {}

# engine splits (tuned against cost model): index -> engine
EVAC_ENGINE = ["act", "dve"] * 6


def _zg(i):
    return i // NG, i % NG


def _host_constants():
    sel33 = np.zeros((128, 33), dtype=np.float32)
    for p in range(128):
        sel33[p, p % 32] = 1.0
    sel33[:, 32] = 1.0
    ones33 = np.ones((128, 33), dtype=np.float32)
    # cw1p: block-ones, col 2*jj+c (c=0,1) selects quadrant jj; cols 8..32 zero
    w1p = np.zeros((128, 32), dtype=np.float32)
    for jj in range(4):
        for c in range(2):
            w1p[32 * jj:32 * jj + 32, 2 * jj + c] = 1.0
    # replicated at partition bases 0/32/64 so lhsT base matches rhs base
    lhsT4 = np.zeros((128, 128), dtype=np.float32)
    for z in range(3):
        for jj in range(4):
            lhsT4[32 * z + jj, 32 * jj:32 * jj + 32] = 1.0
    import ml_dtypes
    cst = np.concatenate([sel33, ones33], axis=1)  # [128, 66] fp32
    cstb = np.concatenate([w1p, lhsT4], axis=1).astype(ml_dtypes.bfloat16)  # [128, 160]
    return {"cst": cst, "cstb": cstb}


def _emit(ctx, tc, x_d, t0_d, t0b_d, cst_d, cstb_d, res_d):
    import concourse.mybir as mybir

    nc = tc.nc
    f32 = mybir.dt.float32
    bf16 = mybir.dt.bfloat16
    Alu = mybir.AluOpType
    Act = mybir.ActivationFunctionType
    AxX = mybir.AxisListType.X

    persist = ctx.enter_context(tc.tile_pool(name="persist", bufs=1))
    scratch = ctx.enter_context(tc.tile_pool(name="scratch", bufs=1))
    stage_p = ctx.enter_context(tc.tile_pool(name="stage", bufs=2))
    p_t0rep = ctx.enter_context(tc.tile_pool(name="p_t0rep", bufs=2, space="PSUM"))
    p_dist = ctx.enter_context(tc.tile_pool(name="p_dist", bufs=2, space="PSUM"))
    p_fin = ctx.enter_context(tc.tile_pool(name="p_fin", bufs=2, space="PSUM"))

    def ptile(shape, tag, dtype=f32):
        return persist.tile(shape, dtype, tag=tag, name=tag)

    # ---- persistent tiles ----
    X = ptile([128, U], "X")
    Xb = ptile([128, U], "Xb", dtype=bf16)
    Xsqb = ptile([128, U], "Xsqb", dtype=bf16)
    T0ccb = ptile([128, NG * CW], "T0ccb", dtype=bf16)
    # masks in the D01 partition layout: P = 4*i + jj, value t0[16384*jj+512*i+q]
    TT01 = ptile([128, 2 * CW], "TT01")    # [T0n | T1n]
    T0n = TT01[:, 0:CW]
    T1n = TT01[:, CW:2 * CW]
    D01 = ptile([128, 2 * CW], "D01")
    CST = ptile([128, 66], "CST")          # [csel | cone] fp32
    csel = CST[:, 0:33]
    cone = CST[:, 33:66]
    CSTB = ptile([128, 160], "CSTB", dtype=bf16)   # [cw1p | clhst4] bf16
    cw1p = CSTB[:, 0:32]
    clhst4 = CSTB[:, 32:160]
    W2B = ptile([128, 32], "W2B", dtype=bf16)
    mnegb = ptile([32, 2], "mnegb", dtype=bf16)
    s0pc = ptile([128, CH], "s0pc")
    rspc = ptile([128, CH], "rspc")
    MISC = ptile([128, 64], "MISC")
    stats3 = MISC[:, 32:35]
    vstats = MISC[:, 35:37]
    stat_f = MISC[0:33, 37:40]
    stat_t = MISC[0:33, 40:43]
    cnts = MISC[0:32, 43:47]               # cols: cnt0s, cnt1s, rc0, rc1
    mraw = MISC[0:32, 47:49]
    mneg = MISC[0:32, 49:51]
    csb = MISC[0:1, 51:53]
    vout = MISC[0:1, 53:55]
    bias01 = MISC[:, 55:57]
    biasdv = MISC[:, 57:58]
    s1col = MISC[0:32, 58:59]

    # ---- loads ----
    x_ap = x_d.ap()
    t0_ap = t0_d.ap()
    nc.gpsimd.memset(biasdv, -DELTA_VAR)
    nc.sync.dma_start(T0n, t0_ap.rearrange("(jj i q) -> i jj q", jj=4, i=32))
    # T0ccb[32*z+jj, 512*g+q] = t0[16384*jj + 512*(11*z+g) + q]  (bf16, exact)
    t0b_ap = t0b_d.ap()
    t0r = t0b_ap.rearrange("(jj u) -> jj u", jj=4)
    for z in range(3):
        span = NG * CW if z < 2 else (CH - 2 * NG) * CW
        nc.scalar.dma_start(
            T0ccb[32 * z:32 * z + 4, 0:span],
            t0r[:, NG * CW * z: NG * CW * z + span],
        )
    nc.sync.dma_start(CST[:], cst_d.ap())
    nc.sync.dma_start(CSTB[:], cstb_d.ap())
    # X in 8 window DMAs [128, 2048] (partition (jj f), cols u-window):
    # one HWDGE hold each instead of 4, alternating the two HWDGE queues
    xr = x_ap.rearrange("f (jj u) -> jj f u", jj=4)
    BW = 4 * CW
    for w in range(U // BW):
        (nc.sync if w % 2 == 0 else nc.scalar).dma_start(
            X[:, w * BW:(w + 1) * BW], xr[:, :, w * BW:(w + 1) * BW])

    # ---- phase 1: per 2048 window ----
    # t0rep replicated across a 4-bank PSUM tile by 4 PE matmuls, then ONE
    # fused multiply+accumulate (scalar_tensor_tensor) gives the masked sum
    # s0 partial; ACT does rs+bf16 cast fused; squares from Xb (bf16 2x).
    for i in range(CH):
        z, g = _zg(i)
        xc = X[:, i * CW:(i + 1) * CW]
        t0rep = p_t0rep.tile([128, CW], f32, tag="t0rep")
        nc.tensor.matmul(
            t0rep[:], clhst4[32 * z:32 * z + 4, :],
            T0ccb[32 * z:32 * z + 4, g * CW:(g + 1) * CW],
            start=True, stop=True,
        )
        scr = scratch.tile([128, CW], f32, tag="scr_ttr", name="scr")
        nc.vector.tensor_tensor(out=scr[:], in0=xc, in1=t0rep[:], op=Alu.mult)
        nc.vector.reduce_sum(s0pc[:, i:i + 1], scr[:], axis=AxX)
    for w in range(U // BW):
        xs = X[:, w * BW:(w + 1) * BW]
        # rs + bf16 cast fused: ACT copy X -> Xb with accumulate
        nc.scalar.activation(
            out=Xb[:, w * BW:(w + 1) * BW], in_=xs, func=Act.Copy,
            accum_out=rspc[:, w:w + 1])
        xq = Xsqb[:, w * BW:(w + 1) * BW]
        eng = ["gps", "dve", "gps", "act", "gps", "dve", "gps", "act"][w]
        if eng == "act":
            nc.scalar.activation(out=xq, in_=xs, func=Act.Square)
        elif eng == "dve":
            xb = Xb[:, w * BW:(w + 1) * BW]
            nc.vector.tensor_tensor(out=xq, in0=xb, in1=xb, op=Alu.mult)
        else:
            nc.gpsimd.tensor_tensor(out=xq, in0=xs, in1=xs, op=Alu.mult)

    # ---- cross-partition finish #1 (per-f sums + totals) ----
    nc.vector.reduce_sum(stats3[:, 0:1], s0pc[:], axis=AxX)
    nc.vector.reduce_sum(stats3[:, 1:2], rspc[:, 0:8], axis=AxX)
    nc.vector.reduce_sum(stats3[:, 2:3], T0n, axis=AxX)
    F1f = p_fin.tile([33, 3], f32, tag="fin")
    nc.tensor.matmul(F1f[:], csel, stats3, start=True, stop=True)
    F1t = p_fin.tile([33, 3], f32, tag="fin")
    nc.tensor.matmul(F1t[:], cone, stats3, start=True, stop=True)
    nc.scalar.copy(stat_f[:], F1f[:])
    nc.scalar.copy(stat_t[:], F1t[:])

    # ---- means (f on partitions 0..31; totals available on every row) ----
    cnt0col = stat_t[0:32, 2:3]
    nc.vector.tensor_scalar(
        out=cnts[:, 0:1], in0=cnt0col, scalar1=1.0, scalar2=None, op0=Alu.max)
    nc.vector.tensor_scalar(
        out=cnts[:, 1:2], in0=cnt0col, scalar1=-1.0, scalar2=float(NLOC),
        op0=Alu.mult, op1=Alu.add)
    nc.vector.tensor_scalar(
        out=cnts[:, 1:2], in0=cnts[:, 1:2], scalar1=1.0, scalar2=None, op0=Alu.max)
    nc.vector.reciprocal(cnts[:, 2:3], cnts[:, 0:1])
    nc.vector.reciprocal(cnts[:, 3:4], cnts[:, 1:2])
    nc.vector.tensor_tensor(
        out=mraw[:, 0:1], in0=stat_f[0:32, 0:1], in1=cnts[:, 2:3], op=Alu.mult)
    nc.vector.tensor_tensor(
        out=s1col, in0=stat_f[0:32, 1:2], in1=stat_f[0:32, 0:1], op=Alu.subtract)
    nc.vector.tensor_tensor(
        out=mraw[:, 1:2], in0=s1col, in1=cnts[:, 3:4], op=Alu.mult)
    nc.vector.tensor_scalar(
        out=mneg, in0=mraw, scalar1=-2.0, scalar2=None, op0=Alu.mult)

    # W2B: block-diagonal -2*m_c weights (bf16) at cols 2*jj+c (others zero)
    nc.vector.tensor_copy(mnegb, mneg)
    nc.gpsimd.memset(W2B[:], 0.0)
    for jj in range(4):
        nc.sync.dma_start(W2B[32 * jj:32 * jj + 32, 2 * jj:2 * jj + 2], mnegb)

    # ||m_c||^2 -> bias01 on all partitions
    mm0 = p_fin.tile([1, 2], f32, tag="fin")
    nc.tensor.matmul(mm0[:], mraw[:, 0:1], mraw, start=True, stop=True)
    mm1 = p_fin.tile([1, 2], f32, tag="fin")
    nc.tensor.matmul(mm1[:], mraw[:, 1:2], mraw, start=True, stop=True)
    nc.scalar.copy(csb[0:1, 0:1], mm0[0:1, 0:1])
    nc.scalar.copy(csb[0:1, 1:2], mm1[0:1, 1:2])
    nc.gpsimd.partition_broadcast(bias01[:], csb, channels=128)

    res_ap = res_d.ap()
    nc.sync.dma_start(res_ap[0:64].rearrange("(c f) -> f c", c=2), mraw)

    # ---- phase 2: dist_c^2 - ||m_c||^2 into PSUM, 3 chunks per bank ----
    d01v = D01.rearrange("(i p) (c q) -> i p c q", i=32, c=2)
    for g in range(NG):
        nz = 3 if g < CH - 2 * NG else 2
        pd = p_dist.tile([128, CW], f32, tag="dist")
        for z in range(nz):
            i = NG * z + g
            nc.tensor.matmul(
                pd[32 * z:32 * z + 32, :], W2B[:],
                Xb[:, i * CW:(i + 1) * CW], start=True, stop=False)
            nc.tensor.matmul(
                pd[32 * z:32 * z + 32, :], cw1p,
                Xsqb[:, i * CW:(i + 1) * CW], start=False, stop=True)
        stg = stage_p.tile([128, CW], f32, tag="stg")
        if EVAC_ENGINE[g % len(EVAC_ENGINE)] == "act":
            nc.scalar.copy(stg[0:32 * nz, :], pd[0:32 * nz, :])
        else:
            nc.vector.tensor_copy(stg[0:32 * nz, :], pd[0:32 * nz, :])
        # widen into D01 with partition P = 4*i + jj (i = 11*z + g):
        # D01[4*i + jj, 512*c + q] = stg[32*z + 2*jj + c, q]; contiguous dst
        for z in range(nz):
            i = NG * z + g
            dst = D01[4 * i:4 * i + 4, :].rearrange("p (c q) -> p c q", c=2)
            (nc.sync if (g + z) % 2 == 0 else nc.scalar).dma_start(
                dst, stg[32 * z:32 * z + 8, :])

    # ---- hinge chain on D01 (in place) ----
    nc.vector.tensor_scalar(
        out=T1n, in0=T0n, scalar1=-1.0, scalar2=1.0,
        op0=Alu.mult, op1=Alu.add)
    for c in range(2):
        half = D01[:, c * CW:(c + 1) * CW]
        nc.vector.tensor_scalar(
            out=half, in0=half, scalar1=bias01[:, c:c + 1], scalar2=0.0,
            op0=Alu.add, op1=Alu.max)
    nc.scalar.activation(out=D01, in_=D01, func=Act.Sqrt)
    nc.scalar.activation(out=D01, in_=D01, func=Act.Relu, bias=biasdv[:, 0:1])
    nc.scalar.activation(out=D01, in_=D01, func=Act.Square)
    for c in range(2):
        scr2 = scratch.tile([128, CW], f32, tag="scr_tail", name="scr2")
        nc.vector.tensor_tensor(
            out=scr2[:], in0=D01[:, c * CW:(c + 1) * CW],
            in1=(T0n if c == 0 else T1n), op=Alu.mult)
        nc.vector.reduce_sum(vstats[:, c:c + 1], scr2[:], axis=AxX)

    # ---- final sums + outputs ----
    F2 = p_fin.tile([1, 2], f32, tag="fin")
    nc.tensor.matmul(F2[:], cone[:, 0:1], vstats, start=True, stop=True)
    nc.scalar.copy(vout, F2[0:1, 0:2])
    nc.sync.dma_start(res_ap[64:67], stat_t[0:1, 0:3])
    nc.sync.dma_start(res_ap[67:69], vout[0:1, 0:2])
    nc.sync.dma_start(res_ap[69:71], csb[0:1, 0:2])


def _build():
    import concourse.bacc as bacc
    import concourse.tile as tile
    import concourse.mybir as mybir

    f32 = mybir.dt.float32
    nc = bacc.Bacc("TRN2", target_bir_lowering=False, debug=False)
    x_d = nc.dram_tensor("x", [NF, NLOC], f32, kind="ExternalInput")
    t0_d = nc.dram_tensor("t0", [NLOC], f32, kind="ExternalInput")
    t0b_d = nc.dram_tensor("t0b", [NLOC], mybir.dt.bfloat16, kind="ExternalInput")
    cst_d = nc.dram_tensor("cst", [128, 66], f32, kind="ExternalInput")
    cstb_d = nc.dram_tensor("cstb", [128, 160], mybir.dt.bfloat16,
                            kind="ExternalInput")
    res_d = nc.dram_tensor("res", [128], f32, kind="ExternalOutput")
    with tile.TileContext(nc) as tc:
        with ExitStack() as ctx:
            _emit(ctx, tc, x_d, t0_d, t0b_d, cst_d, cstb_d, res_d)
    nc.compile()
    return nc


def get_nc():
    if "nc" not in _CACHE:
        _CACHE["nc"] = _build()
    return _CACHE["nc"]


def make_in_maps(input, target):
    consts = _host_constants()
    in_maps = []
    for b in range(input.shape[0]):
        import ml_dtypes
        t0 = np.ascontiguousarray(target[b, 0], dtype=np.float32)
        m = {
            "x": np.ascontiguousarray(input[b], dtype=np.float32),
            "t0": t0,
            "t0b": t0.astype(ml_dtypes.bfloat16),
        }
        m.update(consts)
        in_maps.append(m)
    return in_maps


def combine_host(results, n_clusters):
    """results: list of 8 dicts with 'res' vectors. Returns scalar loss."""
    total = 0.0
    for b in range(BS):
        r = np.asarray(results[b]["res"], dtype=np.float64)
        m0, m1 = r[0:32], r[32:64]
        cnt0 = r[66]
        v0, v1 = r[67], r[68]
        ncb = float(n_clusters[b])
        counts = np.array([cnt0, NLOC - cnt0])
        active = counts > 0
        safe = np.where(active, counts, 1.0)
        c_var = float(np.where(active, np.array([v0, v1]) / safe, 0.0).sum())
        l_var = c_var / ncb
        dn = float(np.sqrt(((m0 - m1) ** 2).sum()))
        c_dist = 2.0 * max(2.0 * DELTA_DIST - dn, 0.0) ** 2
        l_dist = c_dist / (2.0 * ncb * (ncb - 1.0))
        l_reg = 0.5 * (np.sqrt((m0 ** 2).sum()) + np.sqrt((m1 ** 2).sum()))
        total += ALPHA * l_var + BETA * l_dist + GAMMA * l_reg
    return np.float32(total / BS)


def kernel(input, target, n_clusters):
    from concourse import bass_utils

    nc = get_nc()
    in_maps = make_in_maps(np.asarray(input), np.asarray(target))
    br = bass_utils.run_bass_kernel_spmd(nc, in_maps, core_ids=list(range(NCORES)))
    loss = combine_host(br.results, np.asarray(n_clusters))
    return np.array(loss, dtype=np.float32)

